# revision 3
# baseline (speedup 1.0000x reference)
"""Trainium2 Bass kernel for nn_BackupBarrierCBF.

Reference semantics (B=1024, A=64, T=50 unicycle rollout + rect-vs-disc
distance + min-over-horizon + saturation). Crucial subtleties:
  - braking controller: u = (-9*tanh(2*v), 0) => theta is CONSTANT, so
    positions are x0 + cos(theta)*dt*cumsum(v).
  - veh_veh_distance receives traj[..., 0:3] = (x, y, v): the body-frame
    rotation angle is the (time-varying) VELOCITY, not theta.
  - traj slot k holds the state AFTER k+1 steps: position cumsum uses
    v_0..v_k while the stored rotation angle is v_{k+1}.

Layout: a-major / t-inner. VT has 51 t-slots per column (v_0, v_1..v_50);
one masked tensor_tensor_scan computes all inclusive cumsums in a single
instruction. sin/cos of v(t) via ACT Sin; range reduction only for the
first K_red slots (|v| provably <= pi afterwards: while |v|>2.2 each step
shrinks |v| by >= 0.8997, and the map keeps |v| <= pi once there).
cos(x) = sin(pi/2 - |x|) for |x| <= pi.

Sharding: pure data parallel over batch B across 8 cores (128 rows/core,
partition dim = per-core batch).
"""
import numpy as np
import concourse.bass as bass
import concourse.bacc as bacc
import concourse.tile as tile
from concourse import mybir
from concourse.bass_utils import run_bass_kernel_spmd

F32 = mybir.dt.float32
I32 = mybir.dt.int32
OP = mybir.AluOpType
ACT = mybir.ActivationFunctionType

B, A, F = 1024, 64, 15
N_CORES = 8
PB = B // N_CORES          # 128 batch rows per core (partition dim)
T = 50
TS = T + 1                 # t-slots per column in VT/ST (incl v_0 slot)
NC2 = 2 * A                # 128 columns: [ego agents | other agents]
NT = T * A                 # 3200
TWO_PI = float(2.0 * np.pi)

_cache: dict = {}


def _ap(t: bass.AP, extra_offset: int, free_dims: list) -> bass.AP:
    """View into tile t: keep partition dim, replace free dims."""
    return bass.AP(tensor=t.tensor, offset=t.offset + extra_offset,
                   ap=[list(t.ap[0])] + [list(d) for d in free_dims])


def _build(dt_uniform, k_red):
    nc = bacc.Bacc("TRN2", target_bir_lowering=False)
    data = nc.dram_tensor("data", [PB, A * F], F32, kind="ExternalInput")
    out = nc.dram_tensor("out", [PB, A], F32, kind="ExternalOutput")

    with tile.TileContext(nc) as tc:
        with tc.tile_pool(name="pool", bufs=1) as pool:
            # ---------------- load ----------------
            D = pool.tile([PB, A * F], F32)
            nc.sync.dma_start(out=D[:], in_=data[:])

            def fld(k):  # [128, 64] strided view of per-agent field k
                return _ap(D, k, [[F, A]])

            halfpi = pool.tile([PB, 1], F32)
            nc.vector.memset(halfpi[:], float(np.pi / 2))

            # mask for the cumsum scan: 1 everywhere, 0 at each col's t=0
            MASK = pool.tile([PB, NC2 * TS], F32, tag="tMASK")
            nc.gpsimd.memset(MASK[:], 1.0)
            nc.gpsimd.memset(_ap(MASK, 0, [[TS, NC2]]), 0.0)

            # ---------------- constants ----------------
            cons = pool.tile([PB, 12, A], F32)

            def c(i):
                return _ap(cons, i * A, [[1, A]])

            def cb(i):  # broadcast over inner t: [128, 64, T]
                return _ap(cons, i * A, [[1, A], [0, T]])

            C_P0X, C_P0Y = 0, 1
            C_K1X, C_K1Y, C_K2X, C_K2Y = 2, 3, 4, 5
            C_CEDT, C_SEDT, C_CADT, C_SADT = 6, 7, 8, 9
            C_RE, C_RA = 10, 11

            scr = pool.tile([PB, 4, A], F32)

            def s(i):
                return _ap(scr, i * A, [[1, A]])

            ki = pool.tile([PB, A], I32)

            # r_e, r_a first (groups the two Sqrt activations together)
            t2 = s(2)
            nc.vector.tensor_mul(out=t2, in0=fld(8), in1=fld(8))
            nc.vector.tensor_mul(out=s(3), in0=fld(9), in1=fld(9))
            nc.vector.tensor_add(out=t2, in0=t2, in1=s(3))
            nc.scalar.activation(out=c(C_RE), in_=t2, func=ACT.Sqrt, scale=0.25)
            nc.vector.tensor_mul(out=t2, in0=fld(11), in1=fld(11))
            nc.vector.tensor_mul(out=s(3), in0=fld(12), in1=fld(12))
            nc.vector.tensor_add(out=t2, in0=t2, in1=s(3))
            nc.scalar.activation(out=c(C_RA), in_=t2, func=ACT.Sqrt, scale=0.25)

            for dst, ext_f, r_i in ((C_K1X, 8, C_RA), (C_K1Y, 9, C_RA),
                                    (C_K2X, 11, C_RE), (C_K2Y, 12, C_RE)):
                nc.vector.scalar_tensor_tensor(
                    out=c(dst), in0=fld(ext_f), scalar=0.5, in1=c(r_i),
                    op0=OP.mult, op1=OP.add)

            nc.vector.tensor_sub(out=c(C_P0X), in0=fld(4), in1=fld(0))
            nc.vector.tensor_sub(out=c(C_P0Y), in0=fld(5), in1=fld(1))

            def sincos(theta_ap, out_sin, out_cos):
                # range-reduce theta to [-pi, pi], then Sin / shifted Sin
                for want_cos, dst in ((False, out_sin), (True, out_cos)):
                    shift = 0.25 if want_cos else 0.0
                    nc.vector.tensor_scalar(out=s(3), in0=theta_ap,
                                            scalar1=1.0 / TWO_PI, scalar2=shift,
                                            op0=OP.mult, op1=OP.add)
                    nc.vector.tensor_copy(out=ki[:], in_=s(3))
                    nc.vector.tensor_copy(out=s(3), in_=ki[:])
                    nc.vector.scalar_tensor_tensor(
                        out=s(3), in0=s(3), scalar=-TWO_PI, in1=theta_ap,
                        op0=OP.mult, op1=OP.add)
                    nc.scalar.activation(
                        out=dst, in_=s(3), func=ACT.Sin,
                        bias=halfpi[:] if want_cos else 0.0, scale=1.0)

            # cos/sin(theta) * dt  (positions; works for any dt layout)
            sincos(fld(3), c(C_SEDT), c(C_CEDT))
            sincos(fld(7), c(C_SADT), c(C_CADT))
            for i in (C_CEDT, C_SEDT, C_CADT, C_SADT):
                nc.vector.tensor_mul(out=c(i), in0=c(i), in1=fld(14))

            # ---------------- rollout ----------------
            # VT col = veh*64 + agent; slot j at col*TS + j; j=0 is v_0,
            # j = 1..50 are v_1..v_50.
            VT = pool.tile([PB, NC2 * TS], F32, tag="tVT")
            G = pool.tile([PB, NC2], F32)

            def vslot(j):
                return _ap(VT, j, [[TS, NC2]])

            nc.vector.tensor_copy(out=vslot(0), in_=_ap(D, 2, [[4, 2], [F, A]]))

            if dt_uniform is None:
                NDT2 = pool.tile([PB, NC2], F32)
                nc.vector.tensor_scalar_mul(
                    out=NDT2[:], in0=_ap(D, 14, [[0, 2], [F, A]]), scalar1=-9.0)
                for j in range(1, TS):
                    nc.scalar.activation(out=G[:], in_=vslot(j - 1),
                                         func=ACT.Tanh, scale=2.0)
                    nc.vector.tensor_mul(out=G[:], in0=G[:], in1=NDT2[:])
                    nc.vector.tensor_add(out=vslot(j), in0=vslot(j - 1),
                                         in1=G[:])
            else:
                mdt9 = -9.0 * float(dt_uniform)
                for j in range(1, TS):
                    nc.scalar.activation(out=G[:], in_=vslot(j - 1),
                                         func=ACT.Tanh, scale=2.0)
                    nc.vector.scalar_tensor_tensor(
                        out=vslot(j), in0=G[:], scalar=mdt9, in1=vslot(j - 1),
                        op0=OP.mult, op1=OP.add)

            # ---------------- cumsum (one masked scan) ----------------
            # state = mask*state + v  ->  per-col inclusive cumsum S(0..50);
            # S(k) = v_0+..+v_k for k=0..49 (k=50 unused).
            ST = pool.tile([PB, NC2 * TS], F32, tag="tST")
            nc.vector.tensor_tensor_scan(
                out=ST[:], data0=MASK[:], data1=VT[:], initial=0.0,
                op0=OP.mult, op1=OP.add)

            # ---------------- trig of v (angles are v_{k+1}) ----------
            # range-reduce VT slots j=1..k_red in place
            if k_red > 0:
                red_view = _ap(VT, 1, [[TS, NC2], [1, k_red]])
                MS = pool.tile([PB, NC2 * k_red], F32)
                KI2 = pool.tile([PB, NC2 * k_red], I32)
                nc.vector.tensor_scalar_mul(out=MS[:], in0=red_view,
                                            scalar1=1.0 / TWO_PI)
                nc.vector.tensor_copy(out=KI2[:], in_=MS[:])
                nc.vector.tensor_copy(out=MS[:], in_=KI2[:])
                nc.vector.scalar_tensor_tensor(
                    out=red_view, in0=MS[:], scalar=-TWO_PI, in1=red_view,
                    op0=OP.mult, op1=OP.add)

            ang = _ap(VT, 1, [[TS, NC2], [1, T]])       # [128, 128, 50]
            SINV = pool.tile([PB, NC2 * T], F32)        # sin(v), col*50+t
            COSV = pool.tile([PB, NC2 * T], F32)        # cos(v)
            nc.scalar.activation(out=SINV[:], in_=ang, func=ACT.Sin)
            nc.scalar.activation(out=COSV[:], in_=ang, func=ACT.Abs)
            nc.scalar.activation(out=COSV[:], in_=COSV[:], func=ACT.Sin,
                                 bias=halfpi[:], scale=-1.0)

            # ---------------- relative positions ----------------
            # px(t) = p0x + cadt*Sa(t) - cedt*Se(t)   (a-major, t-inner)
            SEv = _ap(ST, 0, [[TS, A], [1, T]])
            SAv = _ap(ST, A * TS, [[TS, A], [1, T]])
            PXY = pool.tile([PB, 2 * NT], F32)
            PX = _ap(PXY, 0, [[1, NT]])
            PY = _ap(PXY, NT, [[1, NT]])
            SCR = pool.tile([PB, 2 * NT], F32, tag="tVT")
            S1 = _ap(SCR, 0, [[1, NT]])
            S2 = _ap(SCR, NT, [[1, NT]])

            nc.vector.tensor_mul(out=S1, in0=SAv, in1=cb(C_CADT))
            nc.vector.tensor_add(out=S1, in0=S1, in1=cb(C_P0X))
            nc.vector.tensor_mul(out=S2, in0=SEv, in1=cb(C_CEDT))
            nc.vector.tensor_sub(out=PX, in0=S1, in1=S2)
            nc.vector.tensor_mul(out=S1, in0=SAv, in1=cb(C_SADT))
            nc.vector.tensor_add(out=S1, in0=S1, in1=cb(C_P0Y))
            nc.vector.tensor_mul(out=S2, in0=SEv, in1=cb(C_SEDT))
            nc.vector.tensor_sub(out=PY, in0=S1, in1=S2)

            # ---------------- body-frame components ----------------
            CE = _ap(COSV, 0, [[1, NT]])
            CA = _ap(COSV, NT, [[1, NT]])
            SE_ = _ap(SINV, 0, [[1, NT]])
            SA_ = _ap(SINV, NT, [[1, NT]])
            R12 = pool.tile([PB, 2 * NT], F32, tag="tMASK")
            R1X = _ap(R12, 0, [[1, NT]])
            R1Y = _ap(R12, NT, [[1, NT]])
            R34 = pool.tile([PB, 2 * NT], F32, tag="tST")
            R2X = _ap(R34, 0, [[1, NT]])
            R2Y = _ap(R34, NT, [[1, NT]])

            nc.vector.tensor_mul(out=S1, in0=CE, in1=PX)
            nc.vector.tensor_mul(out=S2, in0=SE_, in1=PY)
            nc.vector.tensor_add(out=R1X, in0=S1, in1=S2)
            nc.vector.tensor_mul(out=S1, in0=CE, in1=PY)
            nc.vector.tensor_mul(out=S2, in0=SE_, in1=PX)
            nc.vector.tensor_sub(out=R1Y, in0=S1, in1=S2)
            nc.vector.tensor_mul(out=S1, in0=CA, in1=PX)
            nc.vector.tensor_mul(out=S2, in0=SA_, in1=PY)
            nc.vector.tensor_add(out=R2X, in0=S1, in1=S2)  # = -rel2x; |.| ok
            nc.vector.tensor_mul(out=S1, in0=CA, in1=PY)
            nc.vector.tensor_mul(out=S2, in0=SA_, in1=PX)
            nc.vector.tensor_sub(out=R2Y, in0=S2, in1=S1)

            # |rel| (ACT), -k (gpsimd), max-combine, min over t
            for R in (R1X, R1Y, R2X, R2Y):
                nc.scalar.activation(out=R, in_=R, func=ACT.Abs)
            for R, k_i in ((R1X, C_K1X), (R1Y, C_K1Y), (R2X, C_K2X),
                           (R2Y, C_K2Y)):
                nc.gpsimd.tensor_sub(out=R, in0=R, in1=cb(k_i))
            nc.vector.tensor_tensor(out=R1X, in0=R1X, in1=R1Y, op=OP.max)
            nc.vector.tensor_tensor(out=R2X, in0=R2X, in1=R2Y, op=OP.max)
            nc.vector.tensor_tensor(out=R1X, in0=R1X, in1=R2X, op=OP.max)

            H = pool.tile([PB, A], F32)
            nc.vector.tensor_reduce(out=H[:],
                                    in_=_ap(R12, 0, [[T, A], [1, T]]),
                                    axis=mybir.AxisListType.X, op=OP.min)
            OUTT = pool.tile([PB, A], F32)
            nc.scalar.activation(out=H[:], in_=H[:], func=ACT.Tanh, scale=0.1)
            nc.vector.tensor_scalar_mul(out=OUTT[:], in0=H[:], scalar1=5.0)
            nc.sync.dma_start(out=out[:], in_=OUTT[:])

    nc.compile()
    return nc


def _get_nc(dt_uniform, k_red):
    key = ("nc", dt_uniform, k_red)
    if key not in _cache:
        _cache[key] = _build(dt_uniform, k_red)
    return _cache[key]


def _run(data: np.ndarray, trace: bool = False):
    data = np.ascontiguousarray(data, dtype=np.float32)
    assert data.shape == (B, A, F), data.shape
    dt = data[..., 14]
    dt0 = float(dt.flat[0])
    dt_uniform = dt0 if bool(np.all(dt == dt0)) else None
    vmax = float(np.abs(data[..., [2, 6]]).max())
    # slots j >= k_red have |v_j| <= pi (monotone 0.8997/step shrink while
    # |v| > 2.2, and the map keeps |v| <= pi once below)
    k_red = int(min(T, max(0, np.ceil((vmax - np.pi) / 0.8997) + 1)))
    nc = _get_nc(dt_uniform, k_red)
    in_maps = [{"data": data[c * PB:(c + 1) * PB].reshape(PB, A * F)}
               for c in range(N_CORES)]
    res = run_bass_kernel_spmd(nc, in_maps, core_ids=list(range(N_CORES)),
                               trace=trace)
    full = np.concatenate([res.results[c]["out"] for c in range(N_CORES)],
                          axis=0)
    return full, res


def kernel(data: np.ndarray) -> np.ndarray:
    full, _ = _run(data, trace=False)
    return full


# revision 5
# speedup vs baseline: 1.2274x; 1.2274x over previous
"""Trainium2 Bass kernel for nn_BackupBarrierCBF.

Reference semantics (B=1024, A=64, T=50 unicycle rollout + rect-vs-disc
distance + min-over-horizon + saturation). Crucial subtleties:
  - braking controller: u = (-9*tanh(2*v), 0) => theta is CONSTANT, so
    positions are x0 + cos(theta)*dt*cumsum(v).
  - veh_veh_distance receives traj[..., 0:3] = (x, y, v): the body-frame
    rotation angle is the (time-varying) VELOCITY, not theta.
  - traj slot k holds the state AFTER k+1 steps: position cumsum uses
    v_0..v_k while the stored rotation angle is v_{k+1}.

Per-core structure (batch rows on the 128 partitions):
  - 50-step serial v-recurrence on a contiguous ping-pong pair (ACT Tanh +
    DVE scalar_tensor_tensor); per-step DVE side-copies build the col-major
    angle trajectory VT and the cumsum trajectory ST in the rollout's DVE
    idle time (no separate scan pass).
  - sin/cos of v(t) on ACT; range reduction only for the first k_red slots
    (|v| provably <= pi afterwards: while |v|>2.2 each step shrinks |v| by
    >= 0.8997 and the map keeps |v| <= pi once below). cos x = sin(pi/2-|x|).
  - distance phase: ~28 big [128, 64, 50] DVE ops, a-major (unit inner
    stride), per-agent constants broadcast with 0-step APs. abs on ACT.
  - NO gpsimd tensor work: gpsimd shares the DVE SBUF port (measured 2.5x
    DVE slowdown when overlapped - net zero).

Sharding: pure data parallel over batch B across 8 cores (128 rows/core).
"""
import numpy as np
import concourse.bass as bass
import concourse.bacc as bacc
import concourse.tile as tile
from concourse import mybir
from concourse.bass_utils import run_bass_kernel_spmd

F32 = mybir.dt.float32
I32 = mybir.dt.int32
OP = mybir.AluOpType
ACT = mybir.ActivationFunctionType

B, A, F = 1024, 64, 15
N_CORES = 8
PB = B // N_CORES          # 128 batch rows per core (partition dim)
T = 50
NC2 = 2 * A                # 128 columns: [ego agents | other agents]
NT = T * A                 # 3200
TWO_PI = float(2.0 * np.pi)

_cache: dict = {}


def _ap(t: bass.AP, extra_offset: int, free_dims: list) -> bass.AP:
    """View into tile t: keep partition dim, replace free dims."""
    return bass.AP(tensor=t.tensor, offset=t.offset + extra_offset,
                   ap=[list(t.ap[0])] + [list(d) for d in free_dims])


def _build(dt_uniform, k_red):
    nc = bacc.Bacc("TRN2", target_bir_lowering=False)
    data = nc.dram_tensor("data", [PB, A * F], F32, kind="ExternalInput")
    out = nc.dram_tensor("out", [PB, A], F32, kind="ExternalOutput")

    with tile.TileContext(nc) as tc:
        with tc.tile_pool(name="pool", bufs=1) as pool:
            # ---------------- load ----------------
            D = pool.tile([PB, A * F], F32)
            nc.sync.dma_start(out=D[:], in_=data[:])

            def fld(k):  # [128, 64] strided view of per-agent field k
                return _ap(D, k, [[F, A]])

            halfpi = pool.tile([PB, 1], F32)
            nc.vector.memset(halfpi[:], float(np.pi / 2))

            # ---------------- constants ----------------
            cons = pool.tile([PB, 12, A], F32)

            def c(i):
                return _ap(cons, i * A, [[1, A]])

            def cb(i):  # broadcast over inner t: [128, 64, T]
                return _ap(cons, i * A, [[1, A], [0, T]])

            C_P0X, C_P0Y = 0, 1
            C_K1X, C_K1Y, C_K2X, C_K2Y = 2, 3, 4, 5
            C_CEDT, C_SEDT, C_CADT, C_SADT = 6, 7, 8, 9
            C_RE, C_RA = 10, 11

            scr = pool.tile([PB, 4, A], F32)

            def s(i):
                return _ap(scr, i * A, [[1, A]])

            ki = pool.tile([PB, A], I32)

            # r_e, r_a first (groups the two Sqrt activations together)
            t2 = s(2)
            nc.vector.tensor_mul(out=t2, in0=fld(8), in1=fld(8))
            nc.vector.tensor_mul(out=s(3), in0=fld(9), in1=fld(9))
            nc.vector.tensor_add(out=t2, in0=t2, in1=s(3))
            nc.scalar.activation(out=c(C_RE), in_=t2, func=ACT.Sqrt, scale=0.25)
            nc.vector.tensor_mul(out=t2, in0=fld(11), in1=fld(11))
            nc.vector.tensor_mul(out=s(3), in0=fld(12), in1=fld(12))
            nc.vector.tensor_add(out=t2, in0=t2, in1=s(3))
            nc.scalar.activation(out=c(C_RA), in_=t2, func=ACT.Sqrt, scale=0.25)

            for dst, ext_f, r_i in ((C_K1X, 8, C_RA), (C_K1Y, 9, C_RA),
                                    (C_K2X, 11, C_RE), (C_K2Y, 12, C_RE)):
                nc.vector.scalar_tensor_tensor(
                    out=c(dst), in0=fld(ext_f), scalar=0.5, in1=c(r_i),
                    op0=OP.mult, op1=OP.add)

            nc.vector.tensor_sub(out=c(C_P0X), in0=fld(4), in1=fld(0))
            nc.vector.tensor_sub(out=c(C_P0Y), in0=fld(5), in1=fld(1))

            def sincos(theta_ap, out_sin, out_cos):
                # range-reduce theta to [-pi, pi], then Sin / shifted Sin
                for want_cos, dst in ((False, out_sin), (True, out_cos)):
                    shift = 0.25 if want_cos else 0.0
                    nc.vector.tensor_scalar(out=s(3), in0=theta_ap,
                                            scalar1=1.0 / TWO_PI, scalar2=shift,
                                            op0=OP.mult, op1=OP.add)
                    nc.vector.tensor_copy(out=ki[:], in_=s(3))
                    nc.vector.tensor_copy(out=s(3), in_=ki[:])
                    nc.vector.scalar_tensor_tensor(
                        out=s(3), in0=s(3), scalar=-TWO_PI, in1=theta_ap,
                        op0=OP.mult, op1=OP.add)
                    nc.scalar.activation(
                        out=dst, in_=s(3), func=ACT.Sin,
                        bias=halfpi[:] if want_cos else 0.0, scale=1.0)

            # cos/sin(theta) * dt  (positions; works for any dt layout)
            sincos(fld(3), c(C_SEDT), c(C_CEDT))
            sincos(fld(7), c(C_SADT), c(C_CADT))
            for i in (C_CEDT, C_SEDT, C_CADT, C_SADT):
                nc.vector.tensor_mul(out=c(i), in0=c(i), in1=fld(14))

            # ---------------- rollout ----------------
            # Contiguous ping-pong v tiles for the serial chain; per-step
            # side-copy into col-major VT (angle traj, slots j=1..50 at
            # offset j-1) and cumsum accumulate into col-major ST.
            VP = pool.tile([PB, 2, NC2], F32)   # ping-pong v buffers

            def vbuf(j):
                return _ap(VP, (j % 2) * NC2, [[1, NC2]])

            VT = pool.tile([PB, NC2 * T], F32, tag="tVT")  # v_j, j=1..50
            ST = pool.tile([PB, NC2 * T], F32)   # S(k), k=0..49 (cumsums)
            G = pool.tile([PB, NC2], F32)

            def vtslot(j):  # j in 1..50
                return _ap(VT, j - 1, [[T, NC2]])

            def stslot(k):  # k in 0..49
                return _ap(ST, k, [[T, NC2]])

            nc.vector.tensor_copy(out=vbuf(0), in_=_ap(D, 2, [[4, 2], [F, A]]))
            nc.vector.tensor_copy(out=stslot(0), in_=vbuf(0))

            if dt_uniform is None:
                NDT2 = pool.tile([PB, NC2], F32)
                nc.vector.tensor_scalar_mul(
                    out=NDT2[:], in0=_ap(D, 14, [[0, 2], [F, A]]), scalar1=-9.0)

            for j in range(1, T + 1):
                nc.scalar.activation(out=G[:], in_=vbuf(j - 1),
                                     func=ACT.Tanh, scale=2.0)
                if dt_uniform is None:
                    nc.vector.tensor_mul(out=G[:], in0=G[:], in1=NDT2[:])
                    nc.vector.tensor_add(out=vbuf(j), in0=vbuf(j - 1), in1=G[:])
                else:
                    nc.vector.scalar_tensor_tensor(
                        out=vbuf(j), in0=G[:], scalar=-9.0 * float(dt_uniform),
                        in1=vbuf(j - 1), op0=OP.mult, op1=OP.add)
                nc.vector.tensor_copy(out=vtslot(j), in_=vbuf(j))
                if j < T:
                    nc.vector.tensor_add(out=stslot(j), in0=stslot(j - 1),
                                         in1=vbuf(j))

            # ---------------- trig of v (angles are v_{k+1}) ----------
            # range-reduce VT slots 1..k_red in place (they are the only
            # ones that can exceed |v| > pi)
            if k_red > 0:
                red_view = _ap(VT, 0, [[T, NC2], [1, k_red]])
                MS = pool.tile([PB, NC2 * k_red], F32, tag="tPXY")
                KI2 = pool.tile([PB, NC2 * k_red], I32, tag="tSCR")
                nc.vector.tensor_scalar_mul(out=MS[:], in0=red_view,
                                            scalar1=1.0 / TWO_PI)
                nc.vector.tensor_copy(out=KI2[:], in_=MS[:])
                nc.vector.tensor_copy(out=MS[:], in_=KI2[:])
                nc.vector.scalar_tensor_tensor(
                    out=red_view, in0=MS[:], scalar=-TWO_PI, in1=red_view,
                    op0=OP.mult, op1=OP.add)

            SINV = pool.tile([PB, NC2 * T], F32)
            COSV = pool.tile([PB, NC2 * T], F32)
            # order: Abs first, then the two Sins (one ACT table switch)
            nc.scalar.activation(out=COSV[:], in_=VT[:], func=ACT.Abs)
            nc.scalar.activation(out=SINV[:], in_=VT[:], func=ACT.Sin)
            nc.scalar.activation(out=COSV[:], in_=COSV[:], func=ACT.Sin,
                                 bias=halfpi[:], scale=-1.0)

            # ---------------- relative positions ----------------
            SEv = _ap(ST, 0, [[T, A], [1, T]])
            SAv = _ap(ST, A * T, [[T, A], [1, T]])
            PXY = pool.tile([PB, 2 * NT], F32, tag="tPXY")
            PX = _ap(PXY, 0, [[1, NT]])
            PY = _ap(PXY, NT, [[1, NT]])
            SCR = pool.tile([PB, 2 * NT], F32, tag="tSCR")
            S1 = _ap(SCR, 0, [[1, NT]])
            S2 = _ap(SCR, NT, [[1, NT]])

            nc.vector.tensor_mul(out=S1, in0=SAv, in1=cb(C_CADT))
            nc.vector.tensor_add(out=S1, in0=S1, in1=cb(C_P0X))
            nc.vector.tensor_mul(out=S2, in0=SEv, in1=cb(C_CEDT))
            nc.vector.tensor_sub(out=PX, in0=S1, in1=S2)
            nc.vector.tensor_mul(out=S1, in0=SAv, in1=cb(C_SADT))
            nc.vector.tensor_add(out=S1, in0=S1, in1=cb(C_P0Y))
            nc.vector.tensor_mul(out=S2, in0=SEv, in1=cb(C_SEDT))
            nc.vector.tensor_sub(out=PY, in0=S1, in1=S2)

            # ---------------- body-frame components ----------------
            CE = _ap(COSV, 0, [[1, NT]])
            CA = _ap(COSV, NT, [[1, NT]])
            SE_ = _ap(SINV, 0, [[1, NT]])
            SA_ = _ap(SINV, NT, [[1, NT]])
            R12 = pool.tile([PB, 2 * NT], F32, tag="tVT")
            R1X = _ap(R12, 0, [[1, NT]])
            R1Y = _ap(R12, NT, [[1, NT]])
            R34 = pool.tile([PB, 2 * NT], F32)
            R2X = _ap(R34, 0, [[1, NT]])
            R2Y = _ap(R34, NT, [[1, NT]])

            nc.vector.tensor_mul(out=S1, in0=CE, in1=PX)
            nc.vector.tensor_mul(out=S2, in0=SE_, in1=PY)
            nc.vector.tensor_add(out=R1X, in0=S1, in1=S2)
            nc.vector.tensor_mul(out=S1, in0=CE, in1=PY)
            nc.vector.tensor_mul(out=S2, in0=SE_, in1=PX)
            nc.vector.tensor_sub(out=R1Y, in0=S1, in1=S2)
            nc.vector.tensor_mul(out=S1, in0=CA, in1=PX)
            nc.vector.tensor_mul(out=S2, in0=SA_, in1=PY)
            nc.vector.tensor_add(out=R2X, in0=S1, in1=S2)  # = -rel2x; |.| ok
            nc.vector.tensor_mul(out=S1, in0=CA, in1=PY)
            nc.vector.tensor_mul(out=S2, in0=SA_, in1=PX)
            nc.vector.tensor_sub(out=R2Y, in0=S2, in1=S1)

            # |rel| on ACT (paired: one op per R-pair tile), -k, max, min
            nc.scalar.activation(out=R12[:], in_=R12[:], func=ACT.Abs)
            nc.scalar.activation(out=R34[:], in_=R34[:], func=ACT.Abs)
            for R, k_i in ((R1X, C_K1X), (R1Y, C_K1Y), (R2X, C_K2X),
                           (R2Y, C_K2Y)):
                nc.vector.tensor_sub(out=R, in0=R, in1=cb(k_i))
            nc.vector.tensor_tensor(out=R1X, in0=R1X, in1=R1Y, op=OP.max)
            nc.vector.tensor_tensor(out=R2X, in0=R2X, in1=R2Y, op=OP.max)
            nc.vector.tensor_tensor(out=R1X, in0=R1X, in1=R2X, op=OP.max)

            H = pool.tile([PB, A], F32)
            nc.vector.tensor_reduce(out=H[:],
                                    in_=_ap(R12, 0, [[T, A], [1, T]]),
                                    axis=mybir.AxisListType.X, op=OP.min)
            OUTT = pool.tile([PB, A], F32)
            nc.scalar.activation(out=H[:], in_=H[:], func=ACT.Tanh, scale=0.1)
            nc.vector.tensor_scalar_mul(out=OUTT[:], in0=H[:], scalar1=5.0)
            nc.sync.dma_start(out=out[:], in_=OUTT[:])

    nc.compile()
    return nc


def _get_nc(dt_uniform, k_red):
    key = ("nc", dt_uniform, k_red)
    if key not in _cache:
        _cache[key] = _build(dt_uniform, k_red)
    return _cache[key]


def _run(data: np.ndarray, trace: bool = False):
    data = np.ascontiguousarray(data, dtype=np.float32)
    assert data.shape == (B, A, F), data.shape
    dt = data[..., 14]
    dt0 = float(dt.flat[0])
    dt_uniform = dt0 if bool(np.all(dt == dt0)) else None
    vmax = float(np.abs(data[..., [2, 6]]).max())
    # slots j >= k_red have |v_j| <= pi (monotone 0.8997/step shrink while
    # |v| > 2.2, and the map keeps |v| <= pi once below)
    k_red = int(min(T, max(0, np.ceil((vmax - np.pi) / 0.8997) + 1)))
    nc = _get_nc(dt_uniform, k_red)
    in_maps = [{"data": data[c * PB:(c + 1) * PB].reshape(PB, A * F)}
               for c in range(N_CORES)]
    res = run_bass_kernel_spmd(nc, in_maps, core_ids=list(range(N_CORES)),
                               trace=trace)
    full = np.concatenate([res.results[c]["out"] for c in range(N_CORES)],
                          axis=0)
    return full, res


def kernel(data: np.ndarray) -> np.ndarray:
    full, _ = _run(data, trace=False)
    return full
